# revision 42
# baseline (speedup 1.0000x reference)
# Trainium2 Bass kernel for:
#   q = x @ Wq.T + bq ; k = x @ Wk.T + bk ; v = x @ Wv.T + bv
#   g = sigmoid(x @ Wg.T + bg)
#   out = q * cumsum(k*v, axis=seq) * g
#
# Sharding: tensor-parallel split of the 2048 output features across the 8
# cores (256 features each). All ops are per-feature except the d-contraction
# (each core uses the full x) and the cumsum along seq (handled fully on-core
# per (batch, feature)) -> zero cross-core communication.
#
# On-core layout is [e, t] (features on partitions, tokens on the free dim):
#   - linears:  psum[e,t] += W_chunk.T @ x_chunk   (bf16 matmuls, fp32 accum)
#   - bias:     ACT activation Identity with per-partition bias (bf16 out)
#   - sigmoid:  ACT activation with per-partition bias (bf16 out)
#   - cumsum:   DVE tensor_tensor_scan along the free dim (fp32 state/out),
#               chained across token (sub)tiles via initial=prev[:, -1:]
#   - qg mul on the Pool engine, kv/out muls on DVE.
# The host pre-packs x into unit tiles [B, NU, 128p, KC, TT] (loaded in
# 4-chunk quarters) and W into [128p, KC, E] so every DMA row is one long
# contiguous packet (4KB for x quarters, 2KB for W quarters), and transposes
# the [B, E, S] per-core bf16 outputs back to fp32 at the end. The final unit
# is processed in 128-token sub-tiles to shorten the post-matmul drain chain.

from contextlib import ExitStack

import numpy as np
import ml_dtypes

import concourse.bass as bass  # noqa: F401  (bass types referenced via tile/bacc)
import concourse.tile as tile
from concourse import bacc, mybir
from concourse.bass_utils import run_bass_kernel_spmd

N_CORES = 8
B, S, D = 4, 4096, 2048
E = D // N_CORES  # 256 output features per core
TT = 512          # token tile (free dim of psum)
KC = D // 128     # contraction chunks
XJ = 4            # chunks per x quarter-tile
NXJ = KC // XJ    # x quarter-tiles per unit
NU = S // TT      # token tiles per batch
MH = E // 128     # feature halves (psum groups per linear)
MM_DT = mybir.dt.bfloat16
MM_NP = ml_dtypes.bfloat16


def build_nc(b=B, s=S, d=D, e=E, tt=TT, mm_dt=MM_DT, n_cores=N_CORES):
    kc = KC
    nu = NU
    mh = MH
    f32 = mybir.dt.float32
    names = "qkvg"

    nc = bacc.Bacc(
        "TRN2", target_bir_lowering=False, debug=False, num_devices=n_cores
    )
    # x packed on host: X5[b, n, p, j, c, t] = x[b, n*tt+t, (j*4+c)*128+p]
    # (partition-major so a whole unit [128, kc, tt] is one contiguous-row DMA)
    X5 = nc.dram_tensor(
        "X5", [b, nu, 128, NXJ, XJ, tt], mm_dt, kind="ExternalInput"
    ).ap()
    # W packed on host: W5[p, c, e] = W[core_sl][e, c*128+p]
    W5 = {
        x_: nc.dram_tensor(f"W{x_}5", [128, kc, e], mm_dt, kind="ExternalInput").ap()
        for x_ in names
    }
    bias = {
        x_: nc.dram_tensor(f"b{x_}", [e], f32, kind="ExternalInput").ap()
        for x_ in names
    }
    outT = nc.dram_tensor("outT", [b, e, s], mybir.dt.bfloat16, kind="ExternalOutput").ap()

    add = mybir.AluOpType.add
    bypass = mybir.AluOpType.bypass
    mult = mybir.AluOpType.mult
    sigmoid = mybir.ActivationFunctionType.Sigmoid
    identity = mybir.ActivationFunctionType.Identity
    bf16 = mybir.dt.bfloat16

    with tile.TileContext(nc) as tc, ExitStack() as ctx:
        wpool = ctx.enter_context(tc.tile_pool(name="w", bufs=1))
        cpool = ctx.enter_context(tc.tile_pool(name="const", bufs=1))
        xpool = ctx.enter_context(tc.tile_pool(name="x", bufs=3))
        ppool = ctx.enter_context(tc.tile_pool(name="psum", bufs=8, space="PSUM"))
        # bufs must cover the final unit's 4 concurrently-live q/g sub-slice
        # tiles (+1 headroom) — a shallower ring deadlocks tile allocation
        spool = ctx.enter_context(tc.tile_pool(name="work", bufs=5))
        opool = ctx.enter_context(tc.tile_pool(name="out", bufs=3))
        cspool = ctx.enter_context(tc.tile_pool(name="cs", bufs=6))

        # Biases via the SWDGE queue (parallel with the big HWDGE stream):
        # [128, mh], col m = bias[m*128:(m+1)*128]
        b_sb = {}
        for x_ in names:
            t_ = cpool.tile([128, mh], f32, tag=f"b{x_}")
            nc.gpsimd.dma_start(out=t_, in_=bias[x_].rearrange("(m p) -> p m", p=128))
            b_sb[x_] = t_

        def load_x(bi, n, n_dmas=1):
            # one whole-unit tile; n_dmas>1 splits the transfer so early
            # chunks land (and unblock matmuls) sooner
            t_ = xpool.tile([128, kc, tt], mm_dt, tag="xt")
            xsrc = X5[bi][n].rearrange("p j c t -> p (j c) t")
            step = kc // n_dmas
            for c0 in range(0, kc, step):
                nc.sync.dma_start(
                    out=t_[:, c0:c0 + step, :], in_=xsrc[:, c0:c0 + step, :]
                )
            return t_

        # Consumption-ordered prologue: unit (0,0)'s x per-chunk interleaved
        # with Wq chunks (the first chain's operands), then Wk/Wv/Wg, then
        # units (0,1)/(0,2) x.
        w_sb = {}
        for x_ in names:
            t_ = wpool.tile([128, kc, e], mm_dt, tag=f"w{x_}")
            w_sb[x_] = t_

        # single sync-queue prologue in consumption order: x(0,0) quarters
        # interleaved with Wq quarters, then Wk / x(0,1) / Wv / x(0,2) / Wg —
        # so units 1-2's x transfers don't queue behind all twelve W quarters
        # x(0,0) in 2-chunk eighths so the first matmul waits on only 256KB;
        # one Wk quarter after every second x eighth keeps issue pressure low
        # (starting even earlier with 1-chunk slices just moves the wait into
        # PE gaps — the cold DMA stream can't keep pace with the chain).
        # Weight order k,v,q,g matches the matmul chain order.
        x_first = xpool.tile([128, kc, tt], mm_dt, tag="xt")
        x0src = X5[0][0].rearrange("p j c t -> p (j c) t")
        for h in range(kc // 2):
            nc.sync.dma_start(
                out=x_first[:, 2 * h:2 * h + 2, :],
                in_=x0src[:, 2 * h:2 * h + 2, :],
            )
            if h % 2 == 0:
                j = h // 2
                nc.sync.dma_start(
                    out=w_sb["k"][:, j * XJ:(j + 1) * XJ, :],
                    in_=W5["k"][:, j * XJ:(j + 1) * XJ, :],
                )
        for x_ in "vqg":
            for j in range(NXJ):
                nc.sync.dma_start(
                    out=w_sb[x_][:, j * XJ:(j + 1) * XJ, :],
                    in_=W5[x_][:, j * XJ:(j + 1) * XJ, :],
                )

        def emit_unit(bi, n, xt, cs_prev, first_of_seq, vec_sub):
            """Full-width matmul chains for unit (bi, n); the ACT/DVE chain
            runs on vec_sub-wide psum slices (narrow for the final unit so
            the post-matmul drain is short)."""
            # k,v chains first: kv + the serial scan need only those psums,
            # so they hide under the q/g matmul chains
            ps = {}
            for m in range(mh):
                for x_ in "kvqg":
                    p_ = ppool.tile([128, tt], f32, tag="ps")
                    for c in range(kc):
                        nc.tensor.matmul(
                            p_[:],
                            lhsT=w_sb[x_][:, c, m * 128:(m + 1) * 128],
                            rhs=xt[:, c, :],
                            start=(c == 0),
                            stop=(c == kc - 1),
                        )
                    ps[x_, m] = p_

            for m in range(mh):
                # pass 1: k/v bias-adds, kv, scan per sub-slice (two-pass so
                # q/g ACT ops never block later slices' k/v in the queue)
                cs_list = []
                for t0 in range(0, tt, vec_sub):
                    tw = vec_sub
                    sl = slice(t0, t0 + tw)
                    k_sb = spool.tile([128, tw], bf16, tag="k")
                    nc.scalar.activation(
                        k_sb[:], ps["k", m][:, sl], identity,
                        bias=b_sb["k"][:, m:m + 1], scale=1.0,
                    )
                    v_sb = spool.tile([128, tw], bf16, tag="v")
                    nc.scalar.activation(
                        v_sb[:], ps["v", m][:, sl], identity,
                        bias=b_sb["v"][:, m:m + 1], scale=1.0,
                    )
                    kv = spool.tile([128, tw], bf16, tag="kv")
                    nc.vector.tensor_tensor(kv[:], k_sb[:], v_sb[:], mult)
                    cs = cspool.tile([128, tw], f32, tag="cs")
                    init = (0.0 if first_of_seq and t0 == 0
                            else cs_prev[m][:, -1:])
                    nc.vector.tensor_tensor_scan(
                        cs[:], kv[:], kv[:], init, op0=add, op1=bypass
                    )
                    cs_prev[m] = cs
                    cs_list.append(cs)
                # pass 2: q/g bias-adds, qg, output — sub-slices write into
                # one full-width tile so each m issues a single out-DMA
                # (per-slice DMAs cost ~600ns of queue issue time apiece,
                # which is exposed at the kernel tail). All q-hat ops are
                # emitted before any g op: g needs the final matmul, and a
                # q-hat queued behind a g would needlessly wait with it.
                slices = list(range(0, tt, vec_sub))
                o_sb = opool.tile([128, tt], bf16, tag="o")
                q_list, g_list = [], []
                for t0 in slices:
                    sl = slice(t0, t0 + vec_sub)
                    q_sb = spool.tile([128, vec_sub], bf16, tag="q")
                    nc.scalar.activation(
                        q_sb[:], ps["q", m][:, sl], identity,
                        bias=b_sb["q"][:, m:m + 1], scale=1.0,
                    )
                    q_list.append(q_sb)
                for t0 in slices:
                    sl = slice(t0, t0 + vec_sub)
                    g_sb = spool.tile([128, vec_sub], bf16, tag="g")
                    nc.scalar.activation(
                        g_sb[:], ps["g", m][:, sl], sigmoid,
                        bias=b_sb["g"][:, m:m + 1], scale=1.0,
                    )
                    g_list.append(g_sb)
                for i, t0 in enumerate(slices):
                    sl = slice(t0, t0 + vec_sub)
                    qg = spool.tile([128, vec_sub], bf16, tag="qg")
                    nc.gpsimd.tensor_tensor(qg[:], q_list[i][:], g_list[i][:], mult)
                    nc.vector.tensor_tensor(o_sb[:, sl], qg[:], cs_list[i][:], mult)
                nc.sync.dma_start(
                    out=outT[bi][m * 128:(m + 1) * 128, n * tt:(n + 1) * tt],
                    in_=o_sb[:],
                )

        for bi in range(b):
            cs_prev = [None] * mh
            for n in range(nu):
                if bi == 0 and n == 0:
                    xt = x_first
                else:
                    # split loads keep chunks landing ahead of the matmul
                    # stream (a monolithic 2MB DMA starves the PE early on);
                    # steady-state units use halves (fewer, larger packets)
                    xt = load_x(bi, n, n_dmas=4 if bi == 0 else 2)
                last_unit = (bi == b - 1) and (n == nu - 1)
                emit_unit(bi, n, xt, cs_prev, first_of_seq=(n == 0),
                          vec_sub=128 if last_unit else tt)

    nc.compile()
    return nc


_NC_CACHE = {}


def _get_nc():
    if "nc" not in _NC_CACHE:
        _NC_CACHE["nc"] = build_nc()
    return _NC_CACHE["nc"]


# Zeroing low bf16 mantissa bits (RNE) was tested as a PE-power lever; it
# made no measurable difference to the DVFS util limit, so it stays off to
# preserve error margin.
TRUNC_K = 0


def _trunc_bf16_rne(a32, k=TRUNC_K):
    ab = a32.astype(MM_NP)
    if not k:
        return ab
    u = ab.view(np.uint16).astype(np.uint32)
    lsb = (u >> k) & 1
    u = (u + ((1 << (k - 1)) - 1 + lsb)) & (0xFFFF ^ ((1 << k) - 1))
    return u.astype(np.uint16).view(MM_NP)


def make_in_maps(x, Wq, bq, Wk, bk, Wv, bv, Wg, bg, e=E, n_cores=N_CORES):
    # X5[b, n, p, j, c, t] = x[b, n*TT+t, (j*XJ+c)*128+p]
    X5 = _trunc_bf16_rne(np.ascontiguousarray(
        np.asarray(x, dtype=np.float32)
        .reshape(B, NU, TT, NXJ, XJ, 128)
        .transpose(0, 1, 5, 3, 4, 2)
    ))
    Ws = {"q": Wq, "k": Wk, "v": Wv, "g": Wg}
    bs = {"q": bq, "k": bk, "v": bv, "g": bg}
    in_maps = []
    for core in range(n_cores):
        sl = slice(core * e, (core + 1) * e)
        m = {"X5": X5}
        for x_ in "qkvg":
            # W5[p, c, e] = W[sl][e, c*128+p]
            m[f"W{x_}5"] = _trunc_bf16_rne(np.ascontiguousarray(
                np.asarray(Ws[x_][sl, :], dtype=np.float32)
                .T.reshape(KC, 128, e)
                .transpose(1, 0, 2)
            ))
            m[f"b{x_}"] = np.ascontiguousarray(np.asarray(bs[x_][sl], dtype=np.float32))
        in_maps.append(m)
    return in_maps


def gather_out(results, n_cores=N_CORES):
    # each core returns outT [B, E, S] bf16; full out = [B, S, D] fp32
    outs = [r["outT"].astype(np.float32) for r in results]
    full = np.concatenate(outs, axis=1)  # [B, D, S]
    return np.ascontiguousarray(full.transpose(0, 2, 1))


def kernel(x, Wq, bq, Wk, bk, Wv, bv, Wg, bg, **run_kwargs):
    nc = _get_nc()
    in_maps = make_in_maps(x, Wq, bq, Wk, bk, Wv, bv, Wg, bg)
    res = run_bass_kernel_spmd(
        nc, in_maps, core_ids=list(range(N_CORES)), **run_kwargs
    )
    out = gather_out(res.results)
    if run_kwargs:
        _NC_CACHE["last_result"] = res
    return out


# revision 44
# speedup vs baseline: 1.0042x; 1.0042x over previous
# Trainium2 Bass kernel for:
#   q = x @ Wq.T + bq ; k = x @ Wk.T + bk ; v = x @ Wv.T + bv
#   g = sigmoid(x @ Wg.T + bg)
#   out = q * cumsum(k*v, axis=seq) * g
#
# Sharding: tensor-parallel split of the 2048 output features across the 8
# cores (256 features each). All ops are per-feature except the d-contraction
# (each core uses the full x) and the cumsum along seq (handled fully on-core
# per (batch, feature)) -> zero cross-core communication.
#
# On-core layout is [e, t] (features on partitions, tokens on the free dim):
#   - linears:  psum[e,t] += W_chunk.T @ x_chunk   (bf16 matmuls, fp32 accum)
#   - bias:     ACT activation Identity with per-partition bias (bf16 out)
#   - sigmoid:  ACT activation with per-partition bias (bf16 out)
#   - cumsum:   DVE tensor_tensor_scan along the free dim (fp32 state/out),
#               chained across token (sub)tiles via initial=prev[:, -1:]
#   - qg mul on the Pool engine, kv/out muls on DVE.
# The host pre-packs x into unit tiles [B, NU, 128p, KC, TT] (loaded in
# 4-chunk quarters) and W into [128p, KC, E] so every DMA row is one long
# contiguous packet (4KB for x quarters, 2KB for W quarters), and transposes
# the [B, E, S] per-core bf16 outputs back to fp32 at the end. The final unit
# is processed in 128-token sub-tiles to shorten the post-matmul drain chain.

from contextlib import ExitStack

import numpy as np
import ml_dtypes

import concourse.bass as bass  # noqa: F401  (bass types referenced via tile/bacc)
import concourse.tile as tile
from concourse import bacc, mybir
from concourse.bass_utils import run_bass_kernel_spmd

N_CORES = 8
B, S, D = 4, 4096, 2048
E = D // N_CORES  # 256 output features per core
TT = 512          # token tile (free dim of psum)
KC = D // 128     # contraction chunks
XJ = 4            # chunks per x quarter-tile
NXJ = KC // XJ    # x quarter-tiles per unit
NU = S // TT      # token tiles per batch
MH = E // 128     # feature halves (psum groups per linear)
MM_DT = mybir.dt.bfloat16
MM_NP = ml_dtypes.bfloat16


def build_nc(b=B, s=S, d=D, e=E, tt=TT, mm_dt=MM_DT, n_cores=N_CORES):
    kc = KC
    nu = NU
    mh = MH
    f32 = mybir.dt.float32
    names = "qkvg"

    nc = bacc.Bacc(
        "TRN2", target_bir_lowering=False, debug=False, num_devices=n_cores
    )
    # x packed on host: X5[b, n, p, j, c, t] = x[b, n*tt+t, (j*4+c)*128+p]
    # (partition-major so a whole unit [128, kc, tt] is one contiguous-row DMA)
    X5 = nc.dram_tensor(
        "X5", [b, nu, 128, NXJ, XJ, tt], mm_dt, kind="ExternalInput"
    ).ap()
    # W packed on host: W5[p, c, e] = W[core_sl][e, c*128+p]
    W5 = {
        x_: nc.dram_tensor(f"W{x_}5", [128, kc, e], mm_dt, kind="ExternalInput").ap()
        for x_ in names
    }
    bias = {
        x_: nc.dram_tensor(f"b{x_}", [e], f32, kind="ExternalInput").ap()
        for x_ in names
    }
    outT = nc.dram_tensor("outT", [b, e, s], mybir.dt.bfloat16, kind="ExternalOutput").ap()

    add = mybir.AluOpType.add
    bypass = mybir.AluOpType.bypass
    mult = mybir.AluOpType.mult
    sigmoid = mybir.ActivationFunctionType.Sigmoid
    identity = mybir.ActivationFunctionType.Identity
    bf16 = mybir.dt.bfloat16

    with tile.TileContext(nc) as tc, ExitStack() as ctx:
        wpool = ctx.enter_context(tc.tile_pool(name="w", bufs=1))
        cpool = ctx.enter_context(tc.tile_pool(name="const", bufs=1))
        xpool = ctx.enter_context(tc.tile_pool(name="x", bufs=3))
        ppool = ctx.enter_context(tc.tile_pool(name="psum", bufs=8, space="PSUM"))
        spool = ctx.enter_context(tc.tile_pool(name="work", bufs=2))
        opool = ctx.enter_context(tc.tile_pool(name="out", bufs=3))
        cspool = ctx.enter_context(tc.tile_pool(name="cs", bufs=6))

        # Biases via the SWDGE queue (parallel with the big HWDGE stream):
        # [128, mh], col m = bias[m*128:(m+1)*128]
        b_sb = {}
        for x_ in names:
            t_ = cpool.tile([128, mh], f32, tag=f"b{x_}")
            nc.gpsimd.dma_start(out=t_, in_=bias[x_].rearrange("(m p) -> p m", p=128))
            b_sb[x_] = t_

        def load_x(bi, n, n_dmas=1):
            # one whole-unit tile; n_dmas>1 splits the transfer so early
            # chunks land (and unblock matmuls) sooner
            t_ = xpool.tile([128, kc, tt], mm_dt, tag="xt")
            xsrc = X5[bi][n].rearrange("p j c t -> p (j c) t")
            step = kc // n_dmas
            for c0 in range(0, kc, step):
                nc.sync.dma_start(
                    out=t_[:, c0:c0 + step, :], in_=xsrc[:, c0:c0 + step, :]
                )
            return t_

        # Consumption-ordered prologue: unit (0,0)'s x per-chunk interleaved
        # with Wq chunks (the first chain's operands), then Wk/Wv/Wg, then
        # units (0,1)/(0,2) x.
        w_sb = {}
        for x_ in names:
            t_ = wpool.tile([128, kc, e], mm_dt, tag=f"w{x_}")
            w_sb[x_] = t_

        # single sync-queue prologue in consumption order: x(0,0) quarters
        # interleaved with Wq quarters, then Wk / x(0,1) / Wv / x(0,2) / Wg —
        # so units 1-2's x transfers don't queue behind all twelve W quarters
        # x(0,0) in 2-chunk eighths so the first matmul waits on only 256KB;
        # one Wk quarter after every second x eighth keeps issue pressure low
        # (starting even earlier with 1-chunk slices just moves the wait into
        # PE gaps — the cold DMA stream can't keep pace with the chain).
        # Weight order k,v,q,g matches the matmul chain order.
        x_first = xpool.tile([128, kc, tt], mm_dt, tag="xt")
        x0src = X5[0][0].rearrange("p j c t -> p (j c) t")
        for h in range(kc // 2):
            nc.sync.dma_start(
                out=x_first[:, 2 * h:2 * h + 2, :],
                in_=x0src[:, 2 * h:2 * h + 2, :],
            )
            if h % 2 == 0:
                j = h // 2
                nc.sync.dma_start(
                    out=w_sb["k"][:, j * XJ:(j + 1) * XJ, :],
                    in_=W5["k"][:, j * XJ:(j + 1) * XJ, :],
                )
        for x_ in "vqg":
            for j in range(NXJ):
                nc.sync.dma_start(
                    out=w_sb[x_][:, j * XJ:(j + 1) * XJ, :],
                    in_=W5[x_][:, j * XJ:(j + 1) * XJ, :],
                )

        def emit_unit(bi, n, xt, cs_prev, first_of_seq, vec_sub):
            """Full-width matmul chains for unit (bi, n); the ACT/DVE chain
            runs on vec_sub-wide psum slices (narrow for the final unit so
            the post-matmul drain is short)."""
            # k,v chains first: kv + the serial scan need only those psums,
            # so they hide under the q/g matmul chains
            ps = {}
            for m in range(mh):
                for x_ in "kvqg":
                    p_ = ppool.tile([128, tt], f32, tag="ps")
                    for c in range(kc):
                        nc.tensor.matmul(
                            p_[:],
                            lhsT=w_sb[x_][:, c, m * 128:(m + 1) * 128],
                            rhs=xt[:, c, :],
                            start=(c == 0),
                            stop=(c == kc - 1),
                        )
                    ps[x_, m] = p_

            for m in range(mh):
                # pass 1: k/v bias-adds, kv, scan per sub-slice (two-pass so
                # q/g ACT ops never block later slices' k/v in the queue)
                cs_list = []
                for t0 in range(0, tt, vec_sub):
                    tw = vec_sub
                    sl = slice(t0, t0 + tw)
                    k_sb = spool.tile([128, tw], bf16, tag="k")
                    nc.scalar.activation(
                        k_sb[:], ps["k", m][:, sl], identity,
                        bias=b_sb["k"][:, m:m + 1], scale=1.0,
                    )
                    v_sb = spool.tile([128, tw], bf16, tag="v")
                    nc.scalar.activation(
                        v_sb[:], ps["v", m][:, sl], identity,
                        bias=b_sb["v"][:, m:m + 1], scale=1.0,
                    )
                    kv = spool.tile([128, tw], bf16, tag="kv")
                    nc.vector.tensor_tensor(kv[:], k_sb[:], v_sb[:], mult)
                    cs = cspool.tile([128, tw], f32, tag="cs")
                    init = (0.0 if first_of_seq and t0 == 0
                            else cs_prev[m][:, -1:])
                    nc.vector.tensor_tensor_scan(
                        cs[:], kv[:], kv[:], init, op0=add, op1=bypass
                    )
                    cs_prev[m] = cs
                    cs_list.append(cs)
                # pass 2: q/g bias-adds, qg, output — sub-slices write into
                # one full-width tile so each m issues a single out-DMA
                # (per-slice DMAs cost ~600ns of queue issue time apiece,
                # which is exposed at the kernel tail)
                o_sb = opool.tile([128, tt], bf16, tag="o")
                for i, t0 in enumerate(range(0, tt, vec_sub)):
                    tw = vec_sub
                    sl = slice(t0, t0 + tw)
                    q_sb = spool.tile([128, tw], bf16, tag="q")
                    nc.scalar.activation(
                        q_sb[:], ps["q", m][:, sl], identity,
                        bias=b_sb["q"][:, m:m + 1], scale=1.0,
                    )
                    g_sb = spool.tile([128, tw], bf16, tag="g")
                    nc.scalar.activation(
                        g_sb[:], ps["g", m][:, sl], sigmoid,
                        bias=b_sb["g"][:, m:m + 1], scale=1.0,
                    )
                    qg = spool.tile([128, tw], bf16, tag="qg")
                    nc.gpsimd.tensor_tensor(qg[:], q_sb[:], g_sb[:], mult)
                    nc.vector.tensor_tensor(o_sb[:, sl], qg[:], cs_list[i][:], mult)
                nc.sync.dma_start(
                    out=outT[bi][m * 128:(m + 1) * 128, n * tt:(n + 1) * tt],
                    in_=o_sb[:],
                )

        for bi in range(b):
            cs_prev = [None] * mh
            for n in range(nu):
                if bi == 0 and n == 0:
                    xt = x_first
                else:
                    # split loads keep chunks landing ahead of the matmul
                    # stream (a monolithic 2MB DMA starves the PE early on);
                    # steady-state units use halves (fewer, larger packets)
                    xt = load_x(bi, n, n_dmas=4 if bi == 0 else 2)
                last_unit = (bi == b - 1) and (n == nu - 1)
                emit_unit(bi, n, xt, cs_prev, first_of_seq=(n == 0),
                          vec_sub=128 if last_unit else tt)

    nc.compile()
    return nc


_NC_CACHE = {}


def _get_nc():
    if "nc" not in _NC_CACHE:
        _NC_CACHE["nc"] = build_nc()
    return _NC_CACHE["nc"]


# Zeroing low bf16 mantissa bits (RNE) was tested as a PE-power lever; it
# made no measurable difference to the DVFS util limit, so it stays off to
# preserve error margin.
TRUNC_K = 0


def _trunc_bf16_rne(a32, k=TRUNC_K):
    ab = a32.astype(MM_NP)
    if not k:
        return ab
    u = ab.view(np.uint16).astype(np.uint32)
    lsb = (u >> k) & 1
    u = (u + ((1 << (k - 1)) - 1 + lsb)) & (0xFFFF ^ ((1 << k) - 1))
    return u.astype(np.uint16).view(MM_NP)


def make_in_maps(x, Wq, bq, Wk, bk, Wv, bv, Wg, bg, e=E, n_cores=N_CORES):
    # X5[b, n, p, j, c, t] = x[b, n*TT+t, (j*XJ+c)*128+p]
    X5 = _trunc_bf16_rne(np.ascontiguousarray(
        np.asarray(x, dtype=np.float32)
        .reshape(B, NU, TT, NXJ, XJ, 128)
        .transpose(0, 1, 5, 3, 4, 2)
    ))
    Ws = {"q": Wq, "k": Wk, "v": Wv, "g": Wg}
    bs = {"q": bq, "k": bk, "v": bv, "g": bg}
    in_maps = []
    for core in range(n_cores):
        sl = slice(core * e, (core + 1) * e)
        m = {"X5": X5}
        for x_ in "qkvg":
            # W5[p, c, e] = W[sl][e, c*128+p]
            m[f"W{x_}5"] = _trunc_bf16_rne(np.ascontiguousarray(
                np.asarray(Ws[x_][sl, :], dtype=np.float32)
                .T.reshape(KC, 128, e)
                .transpose(1, 0, 2)
            ))
            m[f"b{x_}"] = np.ascontiguousarray(np.asarray(bs[x_][sl], dtype=np.float32))
        in_maps.append(m)
    return in_maps


def gather_out(results, n_cores=N_CORES):
    # each core returns outT [B, E, S] bf16; full out = [B, S, D] fp32
    outs = [r["outT"].astype(np.float32) for r in results]
    full = np.concatenate(outs, axis=1)  # [B, D, S]
    return np.ascontiguousarray(full.transpose(0, 2, 1))


def kernel(x, Wq, bq, Wk, bk, Wv, bv, Wg, bg, **run_kwargs):
    nc = _get_nc()
    in_maps = make_in_maps(x, Wq, bq, Wk, bk, Wv, bv, Wg, bg)
    res = run_bass_kernel_spmd(
        nc, in_maps, core_ids=list(range(N_CORES)), **run_kwargs
    )
    out = gather_out(res.results)
    if run_kwargs:
        _NC_CACHE["last_result"] = res
    return out
